# revision 3
# baseline (speedup 1.0000x reference)
"""Haar DWT (2x2 stride-2 block decomposition) on 8 Trainium2 NeuronCores.

Input x: (32, 3, 512, 512) f32. Outputs (ll, lh, hl, hh): each (32, 3, 256, 256).

Sharding: pure data parallel over the batch dim — 4 images per core, viewed as
12 channel images of 512x512 per core, processed 2 channels per iteration.

Per iteration (2 channels):
  - even rows and odd rows are loaded as two separate SBUF tiles (strided DRAM
    reads with 2 KB runs, contiguous SBUF) so the vertical sum/diff on DVE is
    fully unit-stride;
  - su = e + o and df = o - e on DVE, halved in place on ACT;
  - the horizontal stride-2 column combines produce the four outputs into one
    stacked tile, stored with a single fully contiguous 2 MB DMA (8 KB runs)
    into a [NCH, P, 4, 2, 256] DRAM layout that the host deinterleaves.
"""

import sys

import numpy as np

if "/opt/trn_rl_repo" not in sys.path:
    sys.path.insert(0, "/opt/trn_rl_repo")

from concourse import bacc, mybir
from concourse import tile
from concourse.bass_utils import run_bass_kernel_spmd

N_CORES = 8
B, C, H, W = 32, 3, 512, 512
BPC = B // N_CORES  # images per core
NCH = BPC * C  # channel images per core (12)
P = 128  # SBUF partitions
CPI = 2  # channels per iteration
ITERS = NCH // CPI
HW_OUT = H // 2  # 256

_CACHE = {}


def _build():
    nc = bacc.Bacc("TRN2", target_bir_lowering=False, debug=False)
    f32 = mybir.dt.float32
    # x viewed as [NCH, P, k(row pair in partition), t(row parity), W]
    x = nc.dram_tensor("x", [NCH, P, 2, 2, W], f32, kind="ExternalInput")
    # out viewed as [NCH, P, o(ll/lh/hl/hh), k, W/2]
    out = nc.dram_tensor("out", [NCH, P, 4, 2, HW_OUT], f32, kind="ExternalOutput")
    xa = x.ap()
    oa = out.ap()
    with tile.TileContext(nc) as tc:
        with tc.tile_pool(name="p", bufs=3) as pool:
            for i in range(ITERS):
                c0, c1 = CPI * i, CPI * (i + 1)
                et = pool.tile([P, CPI, 2, W], f32)
                ot = pool.tile([P, CPI, 2, W], f32)
                for n in range(CPI):
                    # per channel: (p, k, w) with k strided over row pairs
                    nc.sync.dma_start(out=et[:, n], in_=xa[c0 + n, :, :, 0, :])
                    nc.sync.dma_start(out=ot[:, n], in_=xa[c0 + n, :, :, 1, :])
                su = pool.tile([P, CPI, 2, W], f32)
                df = pool.tile([P, CPI, 2, W], f32)
                nc.vector.tensor_add(su[:], et[:], ot[:])  # unit-stride
                nc.vector.tensor_sub(df[:], ot[:], et[:])  # unit-stride
                nc.scalar.mul(su[:], su[:], 0.5)
                nc.scalar.mul(df[:], df[:], 0.5)
                sv = su[:].rearrange("p n k (j t) -> p n k j t", t=2)
                dv = df[:].rearrange("p n k (j t) -> p n k j t", t=2)
                outt = pool.tile([P, CPI, 4, 2, HW_OUT], f32)
                nc.vector.tensor_add(
                    outt[:, :, 0], sv[:, :, :, :, 0], sv[:, :, :, :, 1]
                )
                nc.vector.tensor_add(
                    outt[:, :, 1], dv[:, :, :, :, 0], dv[:, :, :, :, 1]
                )
                nc.vector.tensor_sub(
                    outt[:, :, 2], sv[:, :, :, :, 1], sv[:, :, :, :, 0]
                )
                nc.vector.tensor_sub(
                    outt[:, :, 3], dv[:, :, :, :, 1], dv[:, :, :, :, 0]
                )
                # (n, p, o, k, j) -> (p, n, o, k, j); fully contiguous per partition
                nc.sync.dma_start(
                    out=oa[c0:c1].transpose([1, 0, 2, 3, 4]), in_=outt[:]
                )
    nc.compile()
    return nc


def _get_nc():
    if "nc" not in _CACHE:
        _CACHE["nc"] = _build()
    return _CACHE["nc"]


def run(x, **spmd_kwargs):
    """Run the DWT on 8 cores; returns (results_tuple, BassKernelResults)."""
    nc = _get_nc()
    xs = np.ascontiguousarray(np.asarray(x, dtype=np.float32)).reshape(
        N_CORES, NCH, P, 2, 2, W
    )
    in_maps = [{"x": xs[i]} for i in range(N_CORES)]
    res = run_bass_kernel_spmd(nc, in_maps, core_ids=list(range(N_CORES)), **spmd_kwargs)
    # per-core out: (NCH, P, 4, 2, HW_OUT); rows of output image r = 2*p + k
    full = np.stack([res.results[i]["out"] for i in range(N_CORES)])
    # (cores, NCH, P, 4, 2, j) -> (cores, NCH, 4, P, 2, j)
    full = full.transpose(0, 1, 3, 2, 4, 5)
    outs = []
    for o in range(4):
        outs.append(
            np.ascontiguousarray(full[:, :, o]).reshape(B, C, HW_OUT, HW_OUT)
        )
    return tuple(outs), res


def kernel(x):
    out, _ = run(x)
    return out


# revision 4
# speedup vs baseline: 1.0844x; 1.0844x over previous
"""Haar DWT (2x2 stride-2 block decomposition) on 8 Trainium2 NeuronCores.

Input x: (32, 3, 512, 512) f32. Outputs (ll, lh, hl, hh): each (32, 3, 256, 256).

Sharding: pure data parallel over the batch dim — 4 images per core, viewed as
12 channel images of 512x512 per core, one channel per iteration.

Per iteration (one 512x512 channel):
  - one fully contiguous 1 MB load: partition p holds rows 4p..4p+3 (8 KB);
  - su = e + o, df = o - e on DVE via strided row-parity views, halved in
    place on ACT;
  - horizontal stride-2 column combines write all four outputs into one
    stacked tile [P, 4, 2, 256];
  - one fully contiguous 1 MB store (8 KB runs) issued on the second HWDGE
    ring (nc.scalar) so load and store descriptor streams run in parallel.
Host side deinterleaves the [NCH, P, 4, 2, 256] layout with numpy views.
"""

import sys

import numpy as np

if "/opt/trn_rl_repo" not in sys.path:
    sys.path.insert(0, "/opt/trn_rl_repo")

from concourse import bacc, mybir
from concourse import tile
from concourse.bass_utils import run_bass_kernel_spmd

N_CORES = 8
B, C, H, W = 32, 3, 512, 512
BPC = B // N_CORES  # images per core
NCH = BPC * C  # channel images per core (12)
P = 128  # SBUF partitions
HW_OUT = H // 2  # 256

_CACHE = {}


def _build():
    nc = bacc.Bacc("TRN2", target_bir_lowering=False, debug=False)
    f32 = mybir.dt.float32
    # x viewed as [NCH, P, 4 rows, W]; rows of channel: r = 4p + j
    x = nc.dram_tensor("x", [NCH, P, 4, W], f32, kind="ExternalInput")
    # out viewed as [NCH, P, o(ll/lh/hl/hh), k, W/2]; out rows r = 2p + k
    out = nc.dram_tensor("out", [NCH, P, 4, 2, HW_OUT], f32, kind="ExternalOutput")
    xa = x.ap()
    oa = out.ap()
    with tile.TileContext(nc) as tc:
        with tc.tile_pool(name="p", bufs=5) as pool:
            for i in range(NCH):
                xin = pool.tile([P, 4, W], f32)
                nc.sync.dma_start(out=xin[:], in_=xa[i])
                xv = xin[:].rearrange("p (k t) w -> p k t w", t=2)
                e = xv[:, :, 0, :]  # even image rows
                o = xv[:, :, 1, :]  # odd image rows
                su = pool.tile([P, 2, W], f32)
                df = pool.tile([P, 2, W], f32)
                nc.vector.tensor_add(su[:], e, o)
                nc.vector.tensor_sub(df[:], o, e)
                nc.scalar.mul(su[:], su[:], 0.5)
                nc.scalar.mul(df[:], df[:], 0.5)
                sv = su[:].rearrange("p k (j t) -> p k j t", t=2)
                dv = df[:].rearrange("p k (j t) -> p k j t", t=2)
                outt = pool.tile([P, 4, 2, HW_OUT], f32)
                nc.vector.tensor_add(outt[:, 0], sv[:, :, :, 0], sv[:, :, :, 1])
                nc.vector.tensor_add(outt[:, 1], dv[:, :, :, 0], dv[:, :, :, 1])
                nc.vector.tensor_sub(outt[:, 2], sv[:, :, :, 1], sv[:, :, :, 0])
                nc.vector.tensor_sub(outt[:, 3], dv[:, :, :, 1], dv[:, :, :, 0])
                nc.scalar.dma_start(out=oa[i], in_=outt[:])
    nc.compile()
    return nc


def _get_nc():
    if "nc" not in _CACHE:
        _CACHE["nc"] = _build()
    return _CACHE["nc"]


def run(x, **spmd_kwargs):
    """Run the DWT on 8 cores; returns (results_tuple, BassKernelResults)."""
    nc = _get_nc()
    xs = np.ascontiguousarray(np.asarray(x, dtype=np.float32)).reshape(
        N_CORES, NCH, P, 4, W
    )
    in_maps = [{"x": xs[i]} for i in range(N_CORES)]
    res = run_bass_kernel_spmd(nc, in_maps, core_ids=list(range(N_CORES)), **spmd_kwargs)
    # per-core out: (NCH, P, 4, 2, HW_OUT); output image row r = 2*p + k
    full = np.stack([res.results[i]["out"] for i in range(N_CORES)])
    # (cores, NCH, P, 4, 2, j) -> (cores, NCH, 4, P, 2, j)
    full = full.transpose(0, 1, 3, 2, 4, 5)
    outs = []
    for o in range(4):
        outs.append(
            np.ascontiguousarray(full[:, :, o]).reshape(B, C, HW_OUT, HW_OUT)
        )
    return tuple(outs), res


def kernel(x):
    out, _ = run(x)
    return out


# revision 6
# speedup vs baseline: 1.1477x; 1.0584x over previous
"""Haar DWT (2x2 stride-2 block decomposition) on 8 Trainium2 NeuronCores.

Input x: (32, 3, 512, 512) f32. Outputs (ll, lh, hl, hh): each (32, 3, 256, 256).

Sharding: pure data parallel over the batch dim — 4 images per core, viewed as
12 channel images of 512x512 per core, one channel per iteration.

The vertical (row-pair) butterfly runs on the TensorEngine: a constant 128x128
weight matrix W maps 128 image rows to 64 halved row-sums (partitions 0..63)
and 64 halved row-diffs (partitions 64..127) in one matmul per 128-row tile
(4 per channel). The weights are +-0.5 (exact powers of two) and all other
entries are exactly zero, so the result is bit-identical to the fp32 two-op
formulation. The horizontal stride-2 column combine is then just 2 DVE ops per
tile — (even+odd) producing ll|lh stacked over partitions, and (odd-even)
producing hl|hh — reading PSUM, writing a stacked SBUF tile stored with one
fully contiguous 1 MB DMA per channel.

ACT does no elementwise work and issues the store DMAs on the second HWDGE
ring, overlapping the load ring on Sync.
"""

import sys

import numpy as np

if "/opt/trn_rl_repo" not in sys.path:
    sys.path.insert(0, "/opt/trn_rl_repo")

from concourse import bacc, bass, mybir
from concourse import tile
from concourse.bass_utils import run_bass_kernel_spmd

N_CORES = 8
B, C, H, W = 32, 3, 512, 512
BPC = B // N_CORES  # images per core
NCH = BPC * C  # channel images per core (12)
P = 128  # SBUF partitions
NT = H // P  # 128-row tiles per channel (4)
HW_OUT = H // 2  # 256

_CACHE = {}


def _butterfly_weights():
    """W[k, m]: m<64 -> 0.5*(row 2m + row 2m+1); m>=64 -> 0.5*(row 2m'+1 - row 2m')."""
    w = np.zeros((P, P), dtype=np.float32)
    for m in range(64):
        w[2 * m, m] = 0.5
        w[2 * m + 1, m] = 0.5
        w[2 * m, 64 + m] = -0.5
        w[2 * m + 1, 64 + m] = 0.5
    return w


def _build():
    nc = bacc.Bacc("TRN2", target_bir_lowering=False, debug=False)
    f32 = mybir.dt.float32
    # x viewed as [NCH, tile, row-in-tile, W]
    x = nc.dram_tensor("x", [NCH, NT, P, W], f32, kind="ExternalInput")
    w = nc.dram_tensor("w", [P, P], f32, kind="ExternalInput")
    # out[ch, p, t, g, j]: p<64,g=0: ll row 64t+p | p>=64,g=0: lh row 64t+p-64
    #                      p<64,g=1: hl          | p>=64,g=1: hh
    out = nc.dram_tensor("out", [NCH, P, NT, 2, HW_OUT], f32, kind="ExternalOutput")
    xa = x.ap()
    oa = out.ap()
    with tile.TileContext(nc) as tc:
        with (
            tc.tile_pool(name="p", bufs=5) as pool,
            tc.tile_pool(name="w", bufs=1) as wpool,
            tc.tile_pool(name="ps", bufs=8, space=bass.MemorySpace.PSUM) as psum,
        ):
            wt = wpool.tile([P, P], f32)
            nc.sync.dma_start(out=wt[:], in_=w.ap())
            for i in range(NCH):
                xin = pool.tile([P, NT, W], f32)
                # (t, p, w) -> (p, t, w); fully sequential DRAM read
                nc.sync.dma_start(out=xin[:], in_=xa[i].transpose([1, 0, 2]))
                outt = pool.tile([P, NT, 2, HW_OUT], f32)
                for t in range(NT):
                    pt = psum.tile([P, W], f32)
                    nc.tensor.matmul(pt[:], wt[:], xin[:, t, :], start=True, stop=True)
                    pv = pt[:].rearrange("p (j two) -> p j two", two=2)
                    # DVE can read at most one PSUM operand per instruction:
                    # ACT (otherwise idle) stages the even columns into SBUF.
                    cp = pool.tile([P, HW_OUT], f32)
                    nc.scalar.copy(cp[:], pv[:, :, 0])
                    nc.vector.tensor_add(outt[:, t, 0], pv[:, :, 1], cp[:])
                    nc.vector.tensor_sub(outt[:, t, 1], pv[:, :, 1], cp[:])
                nc.scalar.dma_start(out=oa[i], in_=outt[:])
    nc.compile()
    return nc


def _get_nc():
    if "nc" not in _CACHE:
        _CACHE["nc"] = _build()
    return _CACHE["nc"]


def run(x, **spmd_kwargs):
    """Run the DWT on 8 cores; returns (results_tuple, BassKernelResults)."""
    nc = _get_nc()
    xs = np.ascontiguousarray(np.asarray(x, dtype=np.float32)).reshape(
        N_CORES, NCH, NT, P, W
    )
    wmat = _butterfly_weights()
    in_maps = [{"x": xs[i], "w": wmat} for i in range(N_CORES)]
    res = run_bass_kernel_spmd(nc, in_maps, core_ids=list(range(N_CORES)), **spmd_kwargs)
    # per-core out: (NCH, P, NT, 2, HW_OUT)
    full = np.stack([res.results[i]["out"] for i in range(N_CORES)])
    # -> (cores, NCH, NT, P, 2, j): out image row r = 64*t + (p mod 64)
    full = full.transpose(0, 1, 3, 2, 4, 5)
    def expand(sl):  # (cores, NCH, NT, 64, j) -> (B, C, 256, 256)
        return np.ascontiguousarray(sl).reshape(B, C, HW_OUT, HW_OUT)
    ll = expand(full[:, :, :, 0:64, 0, :])
    lh = expand(full[:, :, :, 64:128, 0, :])
    hl = expand(full[:, :, :, 0:64, 1, :])
    hh = expand(full[:, :, :, 64:128, 1, :])
    return (ll, lh, hl, hh), res


def kernel(x):
    out, _ = run(x)
    return out
